# revision 1
# baseline (speedup 1.0000x reference)
"""Causal self-attention + residual + LayerNorm fused Trainium2 kernel.

Problem: B=4, S=2048, D=1024, H=16 heads (hd=64), fp32 in/out.
    qkv = x @ in_proj_w.T + in_proj_b ; causal MHA ; out proj ; y = LN(x + attn_out)

Sharding (zero cross-core communication, 8 NeuronCores):
    core c -> batch b = c % 4, query-group g = c // 4.
    Causal zig-zag balance: g=0 owns query blocks [0:512) and [1536:2048),
    g=1 owns [512:1536). Every core computes full K/V for its batch
    (keys 0:2048), attention only for its own queries, then out-proj +
    residual + LayerNorm for its queries. Outputs are disjoint row sets.

Layout: everything is computed transposed (features on partitions,
tokens on the free axis), which makes every matmul contraction land on
the partition axis with zero on-chip transposes:
    K^T[f,t] / Q^T[f,q] = W^T-tile.T @ x^T        (lhsT = in_proj_w.T tile)
    V[t,f]              = x^T-tile.T @ W^T        (lhsT = x^T tile)
    S^T[k,q]            = K^T-slice.T @ Q^T-slice (contraction = head dim 64,
                                                   two heads packed in the PE
                                                   array via tile_position)
    ctx^T[d,q]          = V-slice.T @ exp(S^T)    (V augmented with a ones
                                                   column -> row 64 of the
                                                   PSUM tile = softmax denom)
    out^T[Do,q]         = out_w.T-tile.T @ ctx^T
    LN stats            = ones.T @ y / ones.T @ y^2 (partition reduction on PE)
Matmuls run in float32r (TF32-like, ~11-bit mantissa, 4x faster than fp32
on the PE; measured end-to-end relerr ~1.5e-4). Softmax skips the max
subtraction (scores ~ N(0,1)) and defers the divide: ctx is normalized by
the reciprocal of the aug-row denominator, broadcast across partitions by
the GPSIMD partition_broadcast op.

The two query-groups differ only in the per-q-tile causal k-tile counts;
both variants are emitted under a tc.If on the partition id, so one SPMD
program serves all 8 cores in a single launch.
"""
import sys

if "/opt/trn_rl_repo" not in sys.path:
    sys.path.insert(0, "/opt/trn_rl_repo")

import numpy as np

B, S, D, H, HD = 4, 2048, 1024, 16, 64
P = 128
QT = 512                      # queries per q-tile (matmul free dim)
NQ = 1024                     # queries per core
NKT = S // P                  # 16 k-tiles per batch
DK = D // P                   # 8 contraction tiles over D
NPLAIN = {0: (0, 12), 1: (4, 8)}   # group -> per-q-tile plain (unmasked) k-tiles

_cache = {}


def _build():
    import concourse.mybir as mybir
    import concourse.tile as tile
    from concourse import bacc
    from concourse.bass import ts
    from concourse.alu_op_type import AluOpType

    f32 = mybir.dt.float32
    f32r = mybir.dt.float32r
    AF = mybir.ActivationFunctionType

    nc = bacc.Bacc("TRN2", target_bir_lowering=False, debug=False, num_devices=8)

    xkv = nc.dram_tensor("xkv", [D, S], f32r, kind="ExternalInput").ap()
    xq = nc.dram_tensor("xq", [D, NQ], f32r, kind="ExternalInput").ap()
    wt = nc.dram_tensor("wt", [D, 3 * D], f32r, kind="ExternalInput").ap()
    wot = nc.dram_tensor("wot", [D, D], f32r, kind="ExternalInput").ap()
    maskd = nc.dram_tensor("maskd", [P, 896], f32r, kind="ExternalInput").ap()
    bqd = nc.dram_tensor("bqd", [D], f32, kind="ExternalInput").ap()
    bkd = nc.dram_tensor("bkd", [D], f32, kind="ExternalInput").ap()
    bvd = nc.dram_tensor("bvd", [D], f32, kind="ExternalInput").ap()
    bod = nc.dram_tensor("bod", [D], f32, kind="ExternalInput").ap()
    gamd = nc.dram_tensor("gamd", [D], f32, kind="ExternalInput").ap()
    betd = nc.dram_tensor("betd", [D], f32, kind="ExternalInput").ap()
    yt = nc.dram_tensor("yt", [D, NQ], f32, kind="ExternalOutput").ap()

    xkv_r = xkv.rearrange("(dk p) t -> p dk t", p=P)
    xq_r = xq.rearrange("(dk p) q -> p dk q", p=P)
    xq_f32 = xq.bitcast(f32).rearrange("(ok p) q -> p ok q", p=P)

    with tile.TileContext(nc) as tc:
        with (
            tc.tile_pool(name="persist", bufs=1) as pers,
            tc.tile_pool(name="proj_ps", bufs=2, space="PSUM") as proj_ps,
        ):
            kt = pers.tile([P, DK, S], f32r)           # K^T       64 KB/part
            msk = pers.tile([P, 896], f32r)            #           3.5 KB
            bia = pers.tile([P, DK, 6], f32)           # bq bk bv bo gam bet
            ones128 = pers.tile([P, 1], f32r)
            eps_t = pers.tile([1, 1], f32)
            nc.vector.memset(eps_t[:], 1e-5)

            nc.sync.dma_start(msk[:], maskd[:])
            for j, src in enumerate((bqd, bkd, bvd, bod, gamd, betd)):
                nc.sync.dma_start(bia[:, :, j], src.rearrange("(f p) -> p f", p=P))
            nc.vector.memset(ones128[:].bitcast(f32), 1.0)

            def bq_(f): return bia[:, f, 0:1]
            def bk_(f): return bia[:, f, 1:2]
            def bo_(f): return bia[:, f, 3:4]
            def gam_(f): return bia[:, f, 4:5]
            def bet_(f): return bia[:, f, 5:6]

            # ---- phase A: K^T projection ------------------------------
            with (
                tc.tile_pool(name="wk", bufs=1) as wkp,
                tc.tile_pool(name="xa", bufs=2) as xap,
            ):
                wk = wkp.tile([P, DK, DK, P], f32r)
                nc.sync.dma_start(
                    wk[:],
                    wt[:, D:2 * D].rearrange("(dk p) (f c) -> p dk f c", p=P, c=P),
                )
                for t in range(S // QT):
                    xc = xap.tile([P, DK, QT], f32r, tag="xa")
                    nc.sync.dma_start(xc[:], xkv_r[:, :, ts(t, QT)])
                    for f in range(DK):
                        ps = proj_ps.tile([P, QT], f32, tag="pp")
                        for dk in range(DK):
                            nc.tensor.matmul(
                                ps[:], wk[:, dk, f, :], xc[:, dk, :],
                                start=(dk == 0), stop=(dk == DK - 1),
                            )
                        nc.vector.tensor_scalar_add(kt[:, f, ts(t, QT)], ps[:], bk_(f))

            with tc.tile_pool(name="vpool", bufs=1) as vp:
                v = vp.tile([P, NKT, H, HD + 1], f32r)   # V aug  65 KB/part
                nc.vector.memset(v[:, :, :, HD].bitcast(f32), 1.0)

                # ---- phase B: V projection (natural orientation) ------
                with (
                    tc.tile_pool(name="wv", bufs=1) as wvp,
                    tc.tile_pool(name="xb", bufs=4) as xbp,
                ):
                    wv = wvp.tile([P, DK, 2, 512], f32r)
                    nc.sync.dma_start(
                        wv[:],
                        wt[:, 2 * D:3 * D].rearrange(
                            "(dk p) (g c) -> p dk g c", p=P, c=512),
                    )
                    for t in range(NKT):
                        xc = xbp.tile([P, DK, P], f32r, tag="xb")
                        nc.sync.dma_start(xc[:], xkv_r[:, :, ts(t, P)])
                        for fg in range(2):
                            ps = proj_ps.tile([P, 512], f32, tag="pp")
                            for dk in range(DK):
                                nc.tensor.matmul(
                                    ps[:], xc[:, dk, :], wv[:, dk, fg, :],
                                    start=(dk == 0), stop=(dk == DK - 1),
                                )
                            for hh in range(8):
                                h = 8 * fg + hh
                                nc.vector.tensor_copy(
                                    v[:, t, h, 0:HD], ps[:, ts(hh, HD)]
                                )

                # ---- phases C-F under the partition-id branch ---------
                with tc.tile_pool(name="qc", bufs=1) as qcp:
                    ctx = qcp.tile([P, DK, QT], f32r)

                    def qproj(qt, qtile):
                        with (
                            tc.tile_pool(name="wq", bufs=2) as wqp,
                            tc.tile_pool(name="xqp", bufs=1) as xqp,
                        ):
                            xc = xqp.tile([P, DK, QT], f32r, tag="xq")
                            nc.sync.dma_start(xc[:], xq_r[:, :, ts(qt, QT)])
                            for f in range(DK):
                                wq = wqp.tile([P, DK, P], f32r, tag="wq")
                                nc.sync.dma_start(
                                    wq[:],
                                    wt[:, ts(f, P)].rearrange(
                                        "(dk p) c -> p dk c", p=P),
                                )
                                ps = proj_ps.tile([P, QT], f32, tag="pp")
                                for dk in range(DK):
                                    nc.tensor.matmul(
                                        ps[:], wq[:, dk, :], xc[:, dk, :],
                                        start=(dk == 0), stop=(dk == DK - 1),
                                    )
                                nc.vector.tensor_scalar_add(
                                    qtile[:, f, :], ps[:], bq_(f))

                    def attn(n_plain, qtile):
                        nk = n_plain + 4
                        with (
                            tc.tile_pool(name="sep", bufs=4) as sep,
                            tc.tile_pool(name="scr", bufs=2) as scr,
                            tc.tile_pool(name="s_ps", bufs=2, space="PSUM") as s_ps,
                            tc.tile_pool(name="c_ps", bufs=2, space="PSUM") as c_ps,
                        ):
                            for hp in range(H // 2):
                                cp0 = c_ps.tile([HD + 1, QT], f32, tag="c0")
                                cp1 = c_ps.tile([HD + 1, QT], f32, tag="c1")
                                for i in range(nk):
                                    sp0 = s_ps.tile([P, QT], f32, tag="s")
                                    sp1 = s_ps.tile([P, QT], f32, tag="s")
                                    nc.tensor.matmul(
                                        sp0[:], kt[0:HD, hp, ts(i, P)],
                                        qtile[0:HD, hp, :], start=True, stop=True,
                                    )
                                    nc.tensor.matmul(
                                        sp1[:], kt[HD:P, hp, ts(i, P)],
                                        qtile[HD:P, hp, :], start=True, stop=True,
                                    )
                                    se0 = sep.tile([P, QT], f32r, tag="se")
                                    se1 = sep.tile([P, QT], f32r, tag="se")
                                    nc.scalar.activation(
                                        se0[:], sp0[:], AF.Exp, scale=0.125)
                                    nc.scalar.activation(
                                        se1[:], sp1[:], AF.Exp, scale=0.125)
                                    if i >= n_plain:
                                        off = 384 - P * (i - n_plain)
                                        nc.vector.tensor_mul(
                                            se0[:], se0[:], msk[:, off:off + QT])
                                        nc.vector.tensor_mul(
                                            se1[:], se1[:], msk[:, off:off + QT])
                                    nc.tensor.matmul(
                                        cp0[:], v[:, i, 2 * hp, :], se0[:],
                                        start=(i == 0), stop=(i == nk - 1),
                                    )
                                    nc.tensor.matmul(
                                        cp1[:], v[:, i, 2 * hp + 1, :], se1[:],
                                        start=(i == 0), stop=(i == nk - 1),
                                    )
                                for j, cp in ((0, cp0), (1, cp1)):
                                    h = 2 * hp + j
                                    po, ft = HD * (h % 2), h // 2
                                    den = scr.tile([1, QT], f32, tag="den")
                                    nc.vector.tensor_copy(den[:], cp[HD:HD + 1, :])
                                    rec = scr.tile([1, QT], f32, tag="rec")
                                    rscr = scr.tile([1, QT], f32, tag="rscr")
                                    nc.vector.reciprocal_approx_accurate(
                                        rec[:], den[:], rscr[:])
                                    bc = scr.tile([HD, QT], f32, tag="bc")
                                    nc.gpsimd.partition_broadcast(bc[:], rec[:])
                                    dst = ctx[po:po + HD, ft, :]
                                    nc.vector.tensor_mul(dst, cp[0:HD, :], bc[:])
                                    nc.vector.tensor_scalar_add(
                                        dst, dst, bia[po:po + HD, ft, 2:3])

                    def outproj_ln(qt):
                        with (
                            tc.tile_pool(name="wo", bufs=3) as wop,
                            tc.tile_pool(name="ep", bufs=1) as ep,
                            tc.tile_pool(name="st_ps", bufs=2, space="PSUM") as st_ps,
                        ):
                            y = ep.tile([P, DK, QT], f32r, tag="y")
                            for o in range(DK):
                                wo = wop.tile([P, DK, P], f32r, tag="wo")
                                nc.sync.dma_start(
                                    wo[:],
                                    wot[:, ts(o, P)].rearrange(
                                        "(dk p) c -> p dk c", p=P),
                                )
                                ps = proj_ps.tile([P, QT], f32, tag="pp")
                                for dk in range(DK):
                                    nc.tensor.matmul(
                                        ps[:], wo[:, dk, :], ctx[:, dk, :],
                                        start=(dk == 0), stop=(dk == DK - 1),
                                    )
                                xr = ep.tile([P, QT], f32, tag="xr", bufs=3)
                                nc.sync.dma_start(xr[:], xq_f32[:, o, ts(qt, QT)])
                                nc.vector.scalar_tensor_tensor(
                                    y[:, o, :], ps[:], bo_(o), xr[:],
                                    AluOpType.add, AluOpType.add,
                                )
                            mu_ps = st_ps.tile([1, QT], f32, tag="mu")
                            for o in range(DK):
                                nc.tensor.matmul(
                                    mu_ps[:], ones128[:], y[:, o, :],
                                    start=(o == 0), stop=(o == DK - 1))
                            ms_ps = st_ps.tile([1, QT], f32, tag="ms")
                            for o in range(DK):
                                ysq = ep.tile([P, QT], f32r, tag="ysq")
                                nc.vector.tensor_mul(
                                    ysq[:], y[:, o, :], y[:, o, :])
                                nc.tensor.matmul(
                                    ms_ps[:], ones128[:], ysq[:],
                                    start=(o == 0), stop=(o == DK - 1))
                            mu = ep.tile([1, QT], f32, tag="mu_sb")
                            nc.scalar.mul(mu[:], mu_ps[:], 1.0 / D)
                            ms = ep.tile([1, QT], f32, tag="ms_sb")
                            nc.scalar.mul(ms[:], ms_ps[:], 1.0 / D)
                            tmp = ep.tile([1, QT], f32, tag="stat_tmp", bufs=2)
                            nc.vector.tensor_mul(tmp[:], mu[:], mu[:])
                            nc.vector.tensor_sub(ms[:], ms[:], tmp[:])  # var
                            sd = ep.tile([1, QT], f32, tag="stat_tmp", bufs=2)
                            nc.scalar.activation(sd[:], ms[:], AF.Sqrt, bias=eps_t[:])
                            rstd = ep.tile([1, QT], f32, tag="rstd")
                            rsc = ep.tile([1, QT], f32, tag="stat_tmp", bufs=2)
                            nc.vector.reciprocal_approx_accurate(
                                rstd[:], sd[:], rsc[:])
                            mu_bc = ep.tile([P, QT], f32, tag="mu_bc")
                            nc.gpsimd.partition_broadcast(mu_bc[:], mu[:])
                            rs_bc = ep.tile([P, QT], f32, tag="rs_bc")
                            nc.gpsimd.partition_broadcast(rs_bc[:], rstd[:])
                            for o in range(DK):
                                t1 = ep.tile([P, QT], f32, tag="t1", bufs=2)
                                nc.vector.tensor_sub(
                                    t1[:], y[:, o, :].bitcast(f32), mu_bc[:])
                                nc.vector.tensor_mul(t1[:], t1[:], rs_bc[:])
                                yo = ep.tile([P, QT], f32, tag="yo", bufs=2)
                                nc.vector.tensor_scalar(
                                    yo[:], t1[:], gam_(o), bet_(o),
                                    AluOpType.mult, AluOpType.add,
                                )
                                nc.sync.dma_start(yt[ts(o, P), ts(qt, QT)], yo[:])

                    def group(g):
                        for qt in range(2):
                            with tc.tile_pool(name="qtp", bufs=1) as qtp:
                                qtile = qtp.tile([P, DK, QT], f32r, tag="qtile")
                                qproj(qt, qtile)
                                attn(NPLAIN[g][qt], qtile)
                            outproj_ln(qt)

                    pid = nc.partition_id()
                    with tc.If(pid < 4) as cmp:
                        group(0)
                    with cmp.Else():
                        group(1)
    nc.compile()
    return nc


def _get_nc():
    if "nc" not in _cache:
        _cache["nc"] = _build()
    return _cache["nc"]


def _prep(x, in_proj_w, in_proj_b, out_w, out_b, gamma, beta):
    x = np.asarray(x, np.float32)
    wt = np.ascontiguousarray(np.asarray(in_proj_w, np.float32).T)
    wot = np.ascontiguousarray(np.asarray(out_w, np.float32).T)
    bqkv = np.asarray(in_proj_b, np.float32)
    bo = np.asarray(out_b, np.float32)
    gam = np.asarray(gamma, np.float32)
    bet = np.asarray(beta, np.float32)
    ku = np.arange(P)[:, None] <= (np.arange(896)[None, :] - 384)
    maskd = ku.astype(np.float32)
    qcols = {
        0: np.r_[0:QT, 3 * QT:4 * QT],
        1: np.r_[QT:3 * QT],
    }
    in_maps = []
    for c in range(8):
        b, g = c % 4, c // 4
        xt = np.ascontiguousarray(x[b].T)
        in_maps.append({
            "xkv": xt,
            "xq": np.ascontiguousarray(xt[:, qcols[g]]),
            "wt": wt,
            "wot": wot,
            "maskd": maskd,
            "bqd": bqkv[0:D], "bkd": bqkv[D:2 * D], "bvd": bqkv[2 * D:3 * D],
            "bod": bo, "gamd": gam, "betd": bet,
        })
    return in_maps, qcols


def _run(in_maps, trace=False, **kw):
    from concourse.bass_utils import run_bass_kernel_spmd

    return run_bass_kernel_spmd(_get_nc(), in_maps, list(range(8)), trace=trace, **kw)


def kernel(x, in_proj_w, in_proj_b, out_w, out_b, gamma, beta):
    in_maps, qcols = _prep(x, in_proj_w, in_proj_b, out_w, out_b, gamma, beta)
    res = _run(in_maps)
    out = np.empty((B, S, D), np.float32)
    for c in range(8):
        out[c % 4, qcols[c // 4]] = res.results[c]["yt"].T
    return out



# revision 4
# speedup vs baseline: 1.3312x; 1.3312x over previous
"""Causal self-attention + residual + LayerNorm fused Trainium2 kernel.

Problem: B=4, S=2048, D=1024, H=16 heads (hd=64), fp32 in/out.
    qkv = x @ in_proj_w.T + in_proj_b ; causal MHA ; out proj ; y = LN(x + attn_out)

Sharding (zero cross-core communication, 8 NeuronCores):
    core c -> batch b = c % 4, query-group g = c // 4.
    Causal zig-zag balance: g=0 owns query blocks [0:512) and [1536:2048),
    g=1 owns [512:1536). Every core computes full K/V for its batch,
    attention + out-proj + residual + LayerNorm for its own queries.

Layout/precision strategy:
  - All matmul operands are bf16 (fp32 PSUM accumulation). On TRN2 the PE
    runs bf16 at the same 1 cycle/row as fp32r, but bf16 halves DMA bytes,
    halves SBUF footprint, and unlocks the DVE 2x perf modes.
  - K^T / Q^T computed transposed (features on partitions); V computed
    natural (tokens on partitions) with an augmented ones-column so the
    probability matmul also produces the softmax denominator in PSUM row 64.
    V carries its projection bias (bias rides through softmax: sum p_i = 1).
  - K and V projections share one x^T tile load and interleave back-to-back
    matmuls so the PE never idles long enough for the HAM clock gate to
    re-throttle it to 1.2 GHz (the dominant cost of the previous version).
  - Scores for a head pair go to [128, 2, 512] PSUM blocks (two k-tiles per
    block); exp is one [128,1024] ACT instruction per block per head. The
    causal mask is a bf16 multiply on the exp'd block (last two blocks only).
  - Out-projection is computed token-oriented (queries on partitions):
    lhsT = ctx^T tile (stationary), moving = out_w^T. LayerNorm then reduces
    along the free axis via bn_stats/bn_aggr and normalizes with per-partition
    tensor_scalar ops - no partition reductions, no broadcasts.
  - out_b is folded into the residual tensor on the host; gamma/beta are
    applied via partition-broadcast tiles built once at init.

The two query-groups differ only in per-q-tile k-tile counts; both variants
are emitted under a tc.If on the partition id (one SPMD program, 8 cores).
"""
import sys

if "/opt/trn_rl_repo" not in sys.path:
    sys.path.insert(0, "/opt/trn_rl_repo")

import numpy as np

B, S, D, H, HD = 4, 2048, 1024, 16, 64
P = 128
QT = 512                       # queries per q-tile
NQ = 1024                      # queries per core
NKT = S // P                   # 16 k-tiles per batch
DK = D // P                    # 8 contraction tiles over D
QSTART = {0: (0, 1536), 1: (512, 1024)}   # group -> q-tile start columns
NKS = {0: (4, 16), 1: (8, 12)}            # group -> per-q-tile k-tile counts

_cache = {}


def _build():
    import concourse.mybir as mybir
    import concourse.tile as tile
    from concourse import bacc
    from concourse.bass import ts
    from concourse.alu_op_type import AluOpType

    f32 = mybir.dt.float32
    bf16 = mybir.dt.bfloat16
    AF = mybir.ActivationFunctionType

    nc = bacc.Bacc("TRN2", target_bir_lowering=False, debug=False, num_devices=8)

    xt = nc.dram_tensor("xt", [D, S], bf16, kind="ExternalInput").ap()
    xrd = nc.dram_tensor("xrd", [NQ, D], f32, kind="ExternalInput").ap()
    wtb = nc.dram_tensor("wtb", [D, 3 * D], bf16, kind="ExternalInput").ap()
    wob = nc.dram_tensor("wob", [D, D], bf16, kind="ExternalInput").ap()
    mskd = nc.dram_tensor("mskd", [P, 4, QT], bf16, kind="ExternalInput").ap()
    bqd = nc.dram_tensor("bqd", [D], f32, kind="ExternalInput").ap()
    bkd = nc.dram_tensor("bkd", [D], f32, kind="ExternalInput").ap()
    bvd = nc.dram_tensor("bvd", [D], f32, kind="ExternalInput").ap()
    gamd = nc.dram_tensor("gamd", [D], f32, kind="ExternalInput").ap()
    betd = nc.dram_tensor("betd", [D], f32, kind="ExternalInput").ap()
    ytd = nc.dram_tensor("ytd", [NQ, D], f32, kind="ExternalOutput").ap()

    xt_r = xt.rearrange("(dk p) t -> p dk t", p=P)
    wk_src = xt  # placeholder to keep linter quiet; real srcs below
    wq_src = wtb[:, 0:D].rearrange("(dk p) (f c) -> p dk f c", p=P, c=P)
    wk_src = wtb[:, D:2 * D].rearrange("(dk p) (f c) -> p dk f c", p=P, c=P)
    wv_src = wtb[:, 2 * D:3 * D].rearrange("(dk p) (g c) -> p dk g c", p=P, c=QT)
    wo_src = wob.rearrange("(dk p) f -> p dk f", p=P)

    with tile.TileContext(nc) as tc:
        with tc.tile_pool(name="persist", bufs=1) as pers:
            kt = pers.tile([P, DK, S], bf16)              # K^T      32 KB/part
            v = pers.tile([P, NKT, H, HD + 2], bf16)      # V aug    33 KB/part
            msk = pers.tile([P, 4, QT], bf16)
            bia = pers.tile([P, DK, 2], f32)              # bq bk per-partition
            gbc = pers.tile([P, D], f32)                  # gamma broadcast
            bbc = pers.tile([P, D], f32)                  # beta broadcast
            eps_t = pers.tile([P, 1], f32)

            nc.vector.memset(eps_t[:], 1e-5)
            nc.sync.dma_start(msk[:], mskd[:])
            nc.sync.dma_start(bia[:, :, 0], bqd.rearrange("(f p) -> p f", p=P))
            nc.sync.dma_start(bia[:, :, 1], bkd.rearrange("(f p) -> p f", p=P))
            nc.vector.memset(v[:, :, :, HD:HD + 1], 1.0)

            # ---- phase AB: K^T and V projection, interleaved ----------
            with (
                tc.tile_pool(name="initrows", bufs=1) as irp,
                tc.tile_pool(name="wk", bufs=1) as wkp,
                tc.tile_pool(name="wv", bufs=1) as wvp,
                tc.tile_pool(name="xa", bufs=3) as xap,
                tc.tile_pool(name="ab_ps", bufs=4, space="PSUM") as abps,
            ):
                rowv = irp.tile([1, D], f32)
                rowg = irp.tile([1, D], f32)
                rowb = irp.tile([1, D], f32)
                bvbc = irp.tile([P, 2, DK, HD], f32)      # V bias bcast
                nc.sync.dma_start(rowv[:], bvd.rearrange("(a d) -> a d", a=1))
                nc.sync.dma_start(rowg[:], gamd.rearrange("(a d) -> a d", a=1))
                nc.sync.dma_start(rowb[:], betd.rearrange("(a d) -> a d", a=1))
                nc.gpsimd.partition_broadcast(bvbc[:], rowv[:])
                nc.gpsimd.partition_broadcast(gbc[:], rowg[:])
                nc.gpsimd.partition_broadcast(bbc[:], rowb[:])

                wk = wkp.tile([P, DK, DK, P], bf16)
                wv = wvp.tile([P, DK, 2, QT], bf16)
                nc.sync.dma_start(wk[:], wk_src)
                nc.sync.dma_start(wv[:], wv_src)

                for t in range(S // QT):
                    xc = xap.tile([P, DK, QT], bf16, tag="xc")
                    nc.sync.dma_start(xc[:], xt_r[:, :, ts(t, QT)])
                    for f in range(DK):
                        ps = abps.tile([P, QT], f32, tag="ab")
                        for dk in range(DK):
                            nc.tensor.matmul(
                                ps[:], wk[:, dk, f, :], xc[:, dk, :],
                                start=(dk == 0), stop=(dk == DK - 1),
                            )
                        nc.vector.tensor_scalar_add(
                            kt[:, f, ts(t, QT)], ps[:], bia[:, f, 1:2])
                    for q4 in range(4):
                        kti = 4 * t + q4
                        for fg in range(2):
                            ps = abps.tile([P, DK, HD], f32, tag="ab")
                            for dk in range(DK):
                                nc.tensor.matmul(
                                    ps[:], xc[:, dk, ts(q4, P)], wv[:, dk, fg, :],
                                    start=(dk == 0), stop=(dk == DK - 1),
                                )
                            nc.vector.tensor_add(
                                v[:, kti, 8 * fg:8 * fg + 8, 0:HD],
                                ps[:], bvbc[:, fg, :, :],
                            )

            # ---- phases C-F: per-query-group work ---------------------
            with (
                tc.tile_pool(name="wq", bufs=2) as wqp,
                tc.tile_pool(name="wot", bufs=1) as wotp,
                tc.tile_pool(name="xq", bufs=2) as qxp,
                tc.tile_pool(name="qt", bufs=2) as qtp,
                tc.tile_pool(name="cx", bufs=2) as cxp,
                tc.tile_pool(name="se", bufs=3) as sep,
                tc.tile_pool(name="scr", bufs=2) as scr,
                tc.tile_pool(name="yx", bufs=2) as yxp,
                tc.tile_pool(name="pp", bufs=2, space="PSUM") as pp,
                tc.tile_pool(name="s_ps", bufs=2, space="PSUM") as sps,
                tc.tile_pool(name="c_ps", bufs=2, space="PSUM") as cps,
            ):
                wot = wotp.tile([P, DK, D], bf16)
                nc.sync.dma_start(wot[:], wo_src)

                def qproj(qpos, qtile):
                    xcq = qxp.tile([P, DK, QT], bf16, tag="xcq")
                    nc.sync.dma_start(xcq[:], xt_r[:, :, qpos:qpos + QT])
                    for f in range(DK):
                        wq = wqp.tile([P, DK, P], bf16, tag="wq")
                        nc.sync.dma_start(wq[:], wq_src[:, :, f, :])
                        ps = pp.tile([P, QT], f32, tag="pp")
                        for dk in range(DK):
                            nc.tensor.matmul(
                                ps[:], wq[:, dk, :], xcq[:, dk, :],
                                start=(dk == 0), stop=(dk == DK - 1),
                            )
                        nc.vector.tensor_scalar_add(
                            qtile[:, f, :], ps[:], bia[:, f, 0:1])

                def attn(nk, qtile, ctxn):
                    nblk = nk // 2
                    for hp in range(H // 2):
                        cp0 = cps.tile([HD + 1, QT], f32, tag="c")
                        cp1 = cps.tile([HD + 1, QT], f32, tag="c")
                        for blk in range(nblk):
                            i0 = 2 * blk
                            sp0 = sps.tile([P, 2, QT], f32, tag="s")
                            sp1 = sps.tile([P, 2, QT], f32, tag="s")
                            for j in range(2):
                                nc.tensor.matmul(
                                    sp0[:, j, :], kt[0:HD, hp, ts(i0 + j, P)],
                                    qtile[0:HD, hp, :], start=True, stop=True,
                                )
                                nc.tensor.matmul(
                                    sp1[:, j, :], kt[HD:P, hp, ts(i0 + j, P)],
                                    qtile[HD:P, hp, :], start=True, stop=True,
                                )
                            se0 = sep.tile([P, 2, QT], bf16, tag="se")
                            se1 = sep.tile([P, 2, QT], bf16, tag="se")
                            nc.scalar.activation(se0[:], sp0[:], AF.Exp, scale=0.125)
                            nc.scalar.activation(se1[:], sp1[:], AF.Exp, scale=0.125)
                            if blk >= nblk - 2:
                                dd = 2 * blk - (nk - 4)
                                nc.vector.tensor_mul(
                                    se0[:], se0[:], msk[:, dd:dd + 2, :])
                                nc.vector.tensor_mul(
                                    se1[:], se1[:], msk[:, dd:dd + 2, :])
                            for j in range(2):
                                st = (blk == 0 and j == 0)
                                sp_ = (blk == nblk - 1 and j == 1)
                                nc.tensor.matmul(
                                    cp0[:], v[:, i0 + j, 2 * hp, 0:HD + 1],
                                    se0[:, j, :], start=st, stop=sp_,
                                )
                                nc.tensor.matmul(
                                    cp1[:], v[:, i0 + j, 2 * hp + 1, 0:HD + 1],
                                    se1[:, j, :], start=st, stop=sp_,
                                )
                        # both heads' denominators on partition 0 (engine
                        # APs may only start at quadrant partition bases)
                        den2 = scr.tile([1, 2, QT], f32, tag="den")
                        nc.vector.tensor_copy(den2[:, 0, :], cp0[HD:HD + 1, :])
                        nc.vector.tensor_copy(den2[:, 1, :], cp1[HD:HD + 1, :])
                        rec2 = scr.tile([1, 2, QT], f32, tag="rec")
                        rsc2 = scr.tile([1, 2, QT], f32, tag="rsc")
                        nc.vector.reciprocal_approx_accurate(
                            rec2[:], den2[:], rsc2[:])
                        bc0 = scr.tile([HD, QT], f32, tag="bc")
                        bc1 = scr.tile([HD, QT], f32, tag="bc")
                        nc.gpsimd.partition_broadcast(bc0[:], rec2[:, 0, :])
                        nc.gpsimd.partition_broadcast(bc1[:], rec2[:, 1, :])
                        nc.vector.tensor_mul(
                            ctxn[0:HD, hp, :], cp0[0:HD, :], bc0[:])
                        nc.vector.tensor_mul(
                            ctxn[HD:P, hp, :], cp1[0:HD, :], bc1[:])

                def outproj_ln(qt, ctxn):
                    for qs in range(4):
                        row0 = qt * QT + qs * P
                        xrt = yxp.tile([P, D], f32, tag="xr")
                        nc.sync.dma_start(xrt[:], xrd[row0:row0 + P, :])
                        y = yxp.tile([P, D], f32, tag="y")
                        for fh in range(2):
                            ps = pp.tile([P, QT], f32, tag="pp")
                            for dk in range(DK):
                                nc.tensor.matmul(
                                    ps[:], ctxn[:, dk, ts(qs, P)],
                                    wot[:, dk, ts(fh, QT)],
                                    start=(dk == 0), stop=(dk == DK - 1),
                                )
                            nc.vector.tensor_add(
                                y[:, ts(fh, QT)], ps[:], xrt[:, ts(fh, QT)])
                        st6 = scr.tile([P, 12], f32, tag="st6")
                        nc.vector.bn_stats(st6[:, 0:6], y[:, 0:QT])
                        nc.vector.bn_stats(st6[:, 6:12], y[:, QT:D])
                        mv = scr.tile([P, 2], f32, tag="mv")
                        nc.vector.bn_aggr(mv[:], st6[:])
                        sd = scr.tile([P, 1], f32, tag="sd")
                        nc.scalar.activation(
                            sd[:], mv[:, 1:2], AF.Sqrt, bias=eps_t[:])
                        rstd = scr.tile([P, 1], f32, tag="rstd")
                        rss = scr.tile([P, 1], f32, tag="rss")
                        nc.vector.reciprocal_approx_accurate(
                            rstd[:], sd[:], rss[:])
                        nc.vector.tensor_scalar(
                            y[:], y[:], mv[:, 0:1], rstd[:],
                            AluOpType.subtract, AluOpType.mult,
                        )
                        nc.vector.tensor_mul(y[:], y[:], gbc[:])
                        nc.vector.tensor_add(y[:], y[:], bbc[:])
                        nc.sync.dma_start(ytd[row0:row0 + P, :], y[:])

                def group(g):
                    for qt in range(2):
                        qtile = qtp.tile([P, DK, QT], bf16, tag="qtile")
                        ctxn = cxp.tile([P, DK, QT], bf16, tag="ctxn")
                        qproj(QSTART[g][qt], qtile)
                        attn(NKS[g][qt], qtile, ctxn)
                        outproj_ln(qt, ctxn)

                pid = nc.partition_id()
                with tc.If(pid < 4) as cmp:
                    group(0)
                with cmp.Else():
                    group(1)
    nc.compile()
    return nc


def _get_nc():
    if "nc" not in _cache:
        _cache["nc"] = _build()
    return _cache["nc"]


def _prep(x, in_proj_w, in_proj_b, out_w, out_b, gamma, beta):
    from ml_dtypes import bfloat16

    x = np.asarray(x, np.float32)
    wtb = np.ascontiguousarray(np.asarray(in_proj_w, np.float32).T).astype(bfloat16)
    wob = np.ascontiguousarray(np.asarray(out_w, np.float32).T).astype(bfloat16)
    bqkv = np.asarray(in_proj_b, np.float32)
    bo = np.asarray(out_b, np.float32)
    gam = np.asarray(gamma, np.float32)
    bet = np.asarray(beta, np.float32)
    pp_, dd_, qq_ = np.arange(P)[:, None, None], np.arange(4)[None, :, None], \
        np.arange(QT)[None, None, :]
    maskd = (qq_ >= dd_ * P + pp_).astype(bfloat16)
    qcols = {
        0: np.r_[0:QT, 3 * QT:4 * QT],
        1: np.r_[QT:3 * QT],
    }
    in_maps = []
    for c in range(8):
        b, g = c % 4, c // 4
        xtb = np.ascontiguousarray(x[b].T).astype(bfloat16)
        xr = np.ascontiguousarray(x[b][qcols[g]]) + bo[None, :]
        in_maps.append({
            "xt": xtb,
            "xrd": xr.astype(np.float32),
            "wtb": wtb,
            "wob": wob,
            "mskd": maskd,
            "bqd": bqkv[0:D], "bkd": bqkv[D:2 * D], "bvd": bqkv[2 * D:3 * D],
            "gamd": gam, "betd": bet,
        })
    return in_maps, qcols


def _run(in_maps, trace=False, **kw):
    from concourse.bass_utils import run_bass_kernel_spmd

    return run_bass_kernel_spmd(_get_nc(), in_maps, list(range(8)), trace=trace, **kw)


def kernel(x, in_proj_w, in_proj_b, out_w, out_b, gamma, beta):
    in_maps, qcols = _prep(x, in_proj_w, in_proj_b, out_w, out_b, gamma, beta)
    res = _run(in_maps)
    out = np.empty((B, S, D), np.float32)
    for c in range(8):
        out[c % 4, qcols[c // 4]] = res.results[c]["ytd"]
    return out
